# revision 14
# baseline (speedup 1.0000x reference)
"""Multi-head attention (QK-LayerNorm, causal) Trainium2 kernel over 8 NeuronCores.

Sharding: tensor-parallel over heads - 2 heads per core. Each core computes
q/k/v projections for its 128 channels, per-head attention for both batches,
and a partial output projection (its 128-channel slice of Wo); the host sums
the 8 partial projections.

Key layout/throughput choices (vs the f32r baseline):
- All DRAM traffic and matmul operands are bf16 (PE rate is identical to
  fp32r at large free-dims, but DMA bytes halve and small-free matmuls avoid
  the fp32r 4x penalty). PSUM accumulation stays f32.
- x is pre-tiled on the host into the exact SBUF layout, so each input tile
  is one DMA of 128 contiguous 2KB rows (4x fewer descriptors).
- Scores for both heads of a (b, qc, kt) step go into one 2-bank PSUM tile
  [128, 1024]; a single Exp drains both, halving ACT access overhead.
- The softmax denominators of both heads are broadcast by one matmul with a
  block-ones [2,128] lhsT and inverted by one reciprocal.
- LayerNorm mean-subtraction is folded into the weights on the host; rstd is
  exp(-0.5*ln(var+eps)) (Exp/Ln share an ACT table; Sqrt does not).
- The denominator is the 65th row of attn@v via a ones-column in V.
- The out-projection of chunk i is software-pipelined into the attention
  kt-loop of chunk i+1 so its PSUM->SBUF drains overlap PE work.
- Engine balance: Square + qkT drain on ACT, reductions/multiplies/copies
  with PSUM operands on DVE, diagonal masking on Pool (no PSUM port there).
"""

import numpy as np
import ml_dtypes

import concourse.bass as bass
import concourse.mybir as mybir
import concourse.tile as tile
from concourse.bass_utils import run_bass_kernel_spmd
from concourse.masks import make_identity

F32 = mybir.dt.float32
F32R = mybir.dt.float32r
BF16 = mybir.dt.bfloat16
BF16NP = ml_dtypes.bfloat16

B, S, D, H = 2, 2048, 1024, 16
DH = D // H          # 64
NCORES = 8
HPC = H // NCORES    # 2 heads per core
CH = HPC * DH        # 128 channels per core
T = B * S            # 4096 tokens
DCH = D // 128       # 8 contraction chunks
TT = T // 128        # 32 token tiles
QW = 512             # q-chunk width
QC = S // QW         # 4 q-chunks per batch
KTB = S // 128       # 16 k-tiles per batch
EPS = 1e-5


def _split_drain_waits(nc):
    """walrus in this env only accepts one sync-wait per instruction;
    hoist extra waits onto preceding single-wait NOPs on the same engine."""
    for f in nc.m.functions:
        for blk in f.blocks:
            new_insts = []
            for inst in blk.instructions:
                si = getattr(inst, "sync_info", None)
                if si is not None and si.on_wait and len(si.on_wait) > 1:
                    waits = list(si.on_wait)
                    for j, w in enumerate(waits[:-1]):
                        new_insts.append(
                            mybir.InstNoOp(
                                name=f"{inst.name}-dwsplit{j}",
                                engine=inst.engine,
                                ins=[],
                                outs=[],
                                sync_info=mybir.SyncInfo(on_wait=[w], on_update=[]),
                            )
                        )
                    si.on_wait = [waits[-1]]
                    inst.sync_info = si
                new_insts.append(inst)
            blk.instructions[:] = new_insts


def _build(use_bias=False):
    nc = bass.Bass("TRN2", target_bir_lowering=False, debug=False)

    # x pre-tiled on host: row (128*t + p) holds x^T[(a*128+p), 128*t:128*(t+1)]
    # flattened over a, i.e. the SBUF tile layout for token-tile t.
    xtl_d = nc.dram_tensor("xtl", [T, D], BF16, kind="ExternalInput")
    wqkvt_d = nc.dram_tensor("wqkvt", [D, 3 * CH], BF16, kind="ExternalInput")
    bqkv_d = (
        nc.dram_tensor("bqkv", [1, 3 * CH], F32, kind="ExternalInput")
        if use_bias
        else None
    )
    wot_d = nc.dram_tensor("wot", [CH, D], BF16, kind="ExternalInput")
    pot_d = nc.dram_tensor("pot", [D, T], BF16, kind="ExternalOutput")

    AF = mybir.ActivationFunctionType
    ALU = mybir.AluOpType

    with tile.TileContext(nc) as tc:
        with (
            tc.tile_pool(name="const", bufs=1) as const_pool,
            tc.tile_pool(name="big", bufs=1) as big,
            tc.tile_pool(name="xt", bufs=6) as xpool,
            tc.tile_pool(name="sq", bufs=3) as sq_pool,
            tc.tile_pool(name="ln", bufs=4) as ln_pool,
            tc.tile_pool(name="qln", bufs=3) as qln_pool,
            tc.tile_pool(name="qkv", bufs=3) as qkv_pool,
            tc.tile_pool(name="ex", bufs=3) as ex_pool,
            tc.tile_pool(name="ao", bufs=3) as ao_pool,
            tc.tile_pool(name="dr", bufs=3) as dr_pool,
            tc.tile_pool(name="po", bufs=4) as po_pool,
        ):
            ident_f = const_pool.tile([128, 128], F32)
            make_identity(nc, ident_f)
            ident = const_pool.tile([128, 128], BF16)
            nc.vector.tensor_copy(out=ident, in_=ident_f)

            epscol = const_pool.tile([128, 1], F32)
            nc.vector.memset(epscol, EPS)

            ones64f = const_pool.tile([1, DH], F32)
            nc.vector.memset(ones64f, 1.0)
            ones64 = const_pool.tile([1, DH], F32R)
            nc.vector.tensor_copy(out=ones64, in_=ones64f)

            wqkv_sb = const_pool.tile([128, DCH, 3 * CH], BF16)
            for d in range(DCH):
                nc.sync.dma_start(
                    out=wqkv_sb[:, d, :],
                    in_=wqkvt_d[128 * d : 128 * (d + 1), :],
                )
            wo_sb = const_pool.tile([128, D], BF16)
            nc.sync.dma_start(out=wo_sb, in_=wot_d[:, :])
            if use_bias:
                bias_sb = const_pool.tile([128, 3 * CH], F32)
                nc.sync.dma_start(
                    out=bias_sb, in_=bqkv_d[0:1, :].to_broadcast([128, 3 * CH])
                )

            # persistent activations
            qkT = big.tile([128, 2, T], BF16)     # [:,0,:]=q^T  [:,1,:]=k^T
            vaug = big.tile([128, TT, 2 * (DH + 1)], BF16)
            ones32 = const_pool.tile([128, TT, 1], BF16)
            nc.vector.memset(ones32, 1.0)
            for h in range(HPC):
                oc = (DH + 1) * h + DH
                nc.vector.tensor_copy(out=vaug[:, :, oc : oc + 1], in_=ones32)

            # ---- Phase 1: q/k/v projection + LN + transposes ----
            psA = tc.alloc_tile_pool(name="psA", bufs=4, space="PSUM")
            psTR = tc.alloc_tile_pool(name="psTR", bufs=4, space="PSUM")
            for t in range(TT):
                xt_sb = xpool.tile([128, DCH, 128], BF16, tag="xt")
                nc.sync.dma_start(
                    out=xt_sb,
                    in_=xtl_d[128 * t : 128 * (t + 1), :].rearrange(
                        "p (a j) -> p a j", j=128
                    ),
                )
                ps = psA.tile([128, 3 * CH], F32, tag="a")
                for d in range(DCH):
                    nc.tensor.matmul(
                        ps,
                        lhsT=xt_sb[:, d, :],
                        rhs=wqkv_sb[:, d, :],
                        start=(d == 0),
                        stop=(d == DCH - 1),
                    )
                if use_bias:
                    qkv = qkv_pool.tile([128, 3 * CH], F32, tag="qkv")
                    nc.vector.tensor_add(out=qkv, in0=ps, in1=bias_sb)
                    src = qkv
                else:
                    src = ps

                # rstd = exp(-0.5*ln(mean(q'^2) + eps)) per (token, head)
                sq = sq_pool.tile([128, 2 * CH], BF16, tag="sq")
                nc.scalar.activation(out=sq, in_=src[:, 0 : 2 * CH], func=AF.Square)
                ssum = ln_pool.tile([128, 4], F32, tag="ssum")
                nc.vector.reduce_sum(
                    out=ssum,
                    in_=sq.rearrange("p (g x) -> p g x", x=DH),
                    axis=mybir.AxisListType.X,
                )
                lnv = ln_pool.tile([128, 4], F32, tag="lnv")
                nc.scalar.activation(
                    out=lnv, in_=ssum, func=AF.Ln, scale=1.0 / DH,
                    bias=epscol[:, :],
                )
                rstd = ln_pool.tile([128, 4], F32, tag="rstd")
                nc.scalar.activation(out=rstd, in_=lnv, func=AF.Exp, scale=-0.5)

                qln = qln_pool.tile([128, 2 * CH], BF16, tag="qln")
                rstd_ap = rstd[:, :]
                rstd_b = bass.AP(
                    tensor=rstd_ap.tensor,
                    offset=rstd_ap.offset,
                    ap=rstd_ap.ap + [[0, DH]],
                )
                nc.vector.tensor_mul(
                    out=qln.rearrange("p (g x) -> p g x", x=DH),
                    in0=src[:, 0 : 2 * CH].rearrange("p (g x) -> p g x", x=DH),
                    in1=rstd_b,
                )

                pst = psTR.tile([128, 256], BF16, tag="t")
                nc.tensor.transpose(pst[:, 0:128], qln[:, 0:CH], ident)
                nc.tensor.transpose(pst[:, 128:256], qln[:, CH : 2 * CH], ident)
                nc.scalar.copy(
                    out=qkT[:, :, 128 * t : 128 * (t + 1)],
                    in_=pst[:, :].rearrange("p (i x) -> p i x", x=128),
                )
                nc.vector.tensor_copy(
                    out=vaug[:, t, :].rearrange("p (h x) -> p h x", x=DH + 1)[
                        :, :, 0:DH
                    ],
                    in_=src[:, 2 * CH : 3 * CH].rearrange("p (h x) -> p h x", x=DH),
                )
            psTR.release()
            psA.release()

            # ---- Phase 2: per-head causal attention + partial out-projection ----
            psS = tc.alloc_tile_pool(name="psS", bufs=2, space="PSUM")
            psO = tc.alloc_tile_pool(name="psO", bufs=2, space="PSUM")
            psX = tc.alloc_tile_pool(name="psX", bufs=2, space="PSUM")

            pending = []

            def flush_one():
                if pending:
                    pending.pop(0)()

            for b in range(B):
                for qc in range(QC):
                    q0 = b * S + qc * QW
                    n_kt = (qc + 1) * (QW // 128)
                    ao = ao_pool.tile([128, QW], BF16, tag="ao")
                    pso0 = psO.tile([DH + 1, QW], F32, tag="o")
                    pso1 = psO.tile([DH + 1, QW], F32, tag="o")
                    pso = [pso0, pso1]
                    for kt in range(n_kt):
                        c0 = max(0, 128 * kt - qc * QW)
                        ps_s = psS.tile([128, 2 * QW], F32, tag="s")
                        for h in range(HPC):
                            nc.tensor.matmul(
                                ps_s[:, h * QW + c0 : (h + 1) * QW],
                                lhsT=qkT[
                                    DH * h : DH * (h + 1),
                                    1,
                                    b * S + 128 * kt : b * S + 128 * (kt + 1),
                                ],
                                rhs=qkT[DH * h : DH * (h + 1), 0, q0 + c0 : q0 + QW],
                                start=True,
                                stop=True,
                            )
                        ex = ex_pool.tile([128, 2 * QW], BF16, tag="ex")
                        exv = ex[:, :].rearrange("p (i x) -> p i x", x=QW)
                        psv = ps_s[:, :].rearrange("p (i x) -> p i x", x=QW)
                        nc.scalar.activation(
                            out=exv[:, :, c0:QW],
                            in_=psv[:, :, c0:QW],
                            func=AF.Exp,
                            scale=1.0 / np.sqrt(DH),
                        )
                        d0 = 128 * kt - qc * QW
                        if d0 >= 0:
                            # diagonal tile: zero exp(s) where k > q (both heads)
                            nc.gpsimd.affine_select(
                                out=exv[:, :, d0 : d0 + 128],
                                in_=exv[:, :, d0 : d0 + 128],
                                compare_op=ALU.is_ge,
                                fill=0.0,
                                base=0,
                                pattern=[[0, 2], [1, 128]],
                                channel_multiplier=-1,
                            )
                        for h in range(HPC):
                            nc.tensor.matmul(
                                pso[h][:, c0:QW],
                                lhsT=vaug[
                                    :, b * KTB + kt, (DH + 1) * h : (DH + 1) * (h + 1)
                                ],
                                rhs=ex[:, h * QW + c0 : (h + 1) * QW],
                                start=(kt == 0),
                                stop=(kt == n_kt - 1),
                            )
                        flush_one()

                    # softmax denominators: per-head broadcast matmul +
                    # reciprocal, then normalize into ao
                    for h in range(HPC):
                        dnh = dr_pool.tile([1, QW], F32R, tag="dn")
                        nc.vector.tensor_copy(out=dnh, in_=pso[h][DH : DH + 1, :])
                        psb = psX.tile([DH, QW], F32, tag="x")
                        nc.tensor.matmul(
                            psb, lhsT=ones64, rhs=dnh, start=True, stop=True
                        )
                        rdb = dr_pool.tile([DH, QW], F32, tag="rdb")
                        nc.vector.reciprocal(out=rdb, in_=psb)
                        nc.vector.tensor_mul(
                            out=ao[DH * h : DH * (h + 1), :],
                            in0=pso[h][0:DH, :],
                            in1=rdb,
                        )

                    # out-projection, interleaved into the next chunk's kt-loop
                    for dcp in range(DCH // 2):
                        def step(dcp=dcp, ao=ao, q0=q0):
                            po_sb = po_pool.tile([128, 2, QW], BF16, tag="po")
                            for i in range(2):
                                dc = 2 * dcp + i
                                ps_po = psX.tile([128, QW], F32, tag="x")
                                nc.tensor.matmul(
                                    ps_po,
                                    lhsT=wo_sb[:, 128 * dc : 128 * (dc + 1)],
                                    rhs=ao,
                                    start=True,
                                    stop=True,
                                )
                                nc.vector.tensor_copy(
                                    out=po_sb[:, i, :], in_=ps_po
                                )
                            dc0 = 2 * dcp
                            nc.sync.dma_start(
                                out=pot_d[
                                    128 * dc0 : 128 * (dc0 + 2), q0 : q0 + QW
                                ].rearrange("(i p) q -> p i q", p=128),
                                in_=po_sb,
                            )
                        pending.append(step)
            while pending:
                pending.pop(0)()

            psX.release()
            psO.release()
            psS.release()

    _split_drain_waits(nc)
    return nc


_NC_CACHE = {}


def _get_nc(use_bias=False):
    if use_bias not in _NC_CACHE:
        _NC_CACHE[use_bias] = _build(use_bias)
    return _NC_CACHE[use_bias]


def _prep_inputs(x, Wq, bq, Wk, bk, Wv, bv, Wo):
    xT = np.ascontiguousarray(x.reshape(T, D).T).astype(np.float32)  # [D, T]
    # SBUF tile layout: row (128t+p) = x^T[a*128+p, 128t+j] flattened over (a, j)
    xtl = (
        xT.reshape(DCH, 128, TT, 128)
        .transpose(2, 1, 0, 3)
        .reshape(T, D)
        .astype(BF16NP)
    )
    in_maps = []
    for c in range(NCORES):
        sl = slice(CH * c, CH * (c + 1))
        wq_c = np.array(Wq[sl, :], dtype=np.float32)
        bq_c = np.array(bq[sl], dtype=np.float32)
        wk_c = np.array(Wk[sl, :], dtype=np.float32)
        bk_c = np.array(bk[sl], dtype=np.float32)
        # fold the LayerNorm mean-subtraction (a linear map) into W and b
        for h in range(HPC):
            blk = slice(DH * h, DH * (h + 1))
            wq_c[blk, :] -= wq_c[blk, :].mean(axis=0, keepdims=True)
            bq_c[blk] -= bq_c[blk].mean()
            wk_c[blk, :] -= wk_c[blk, :].mean(axis=0, keepdims=True)
            bk_c[blk] -= bk_c[blk].mean()
        wv_c = np.array(Wv[sl, :], dtype=np.float32)
        bv_c = np.array(bv[sl], dtype=np.float32)
        wqkvt = np.ascontiguousarray(
            np.concatenate([wq_c, wk_c, wv_c], axis=0).T
        ).astype(BF16NP)
        bqkv = np.concatenate([bq_c, bk_c, bv_c])[None, :].astype(np.float32)
        wot = np.ascontiguousarray(Wo[:, sl].T).astype(BF16NP)
        in_maps.append({"xtl": xtl, "wqkvt": wqkvt, "bqkv": bqkv, "wot": wot})
    return in_maps


def kernel(x, mask, Wq, bq, Wk, bk, Wv, bv, Wo, bo, _trace=False):
    x = np.asarray(x, dtype=np.float32)
    in_maps = _prep_inputs(
        x,
        np.asarray(Wq),
        np.asarray(bq),
        np.asarray(Wk),
        np.asarray(bk),
        np.asarray(Wv),
        np.asarray(bv),
        np.asarray(Wo),
    )
    use_bias = bool(
        np.any(np.asarray(bq)) or np.any(np.asarray(bk)) or np.any(np.asarray(bv))
    )
    if not use_bias:
        for m in in_maps:
            del m["bqkv"]
    nc = _get_nc(use_bias)
    res = run_bass_kernel_spmd(
        nc, in_maps, core_ids=list(range(NCORES)), trace=_trace
    )
    pot = np.zeros((D, T), np.float64)
    for c in range(NCORES):
        pot += res.results[c]["pot"].astype(np.float64)
    out = pot.T.astype(np.float32) + np.asarray(bo, dtype=np.float32)[None, :]
    out = out.reshape(B, S, D)
    if _trace:
        return out, res
    return out
